# revision 1
# baseline (speedup 1.0000x reference)
"""Trainium2 Bass kernel for nn_KC_Avg_Embedding (multi-hot averaged embedding).

Computes, for multi-hot indicator vectors x[b,s,:] over a vocabulary of 1024:
    out[b,s,:] = (x[b,s,:] @ E) / max(sum(x[b,s,:]), 1)

Strategy (data-parallel over 8 NeuronCores, batch-sharded):
  - Each core gets rows = (B/8)*S = 3200 rows of x [3200, 1024] fp32 plus the
    full embedding matrix E [1024, 128] fp32.
  - x is DMA'd in with an fp32->bf16 cast (SWDGE); x is 0/1 so bf16 is exact.
  - Each [128, 128] block of x is transposed on the TensorEngine (via
    identity-matmul transpose) so the vocab dim lands on partitions.
  - E is split on-chip into bf16 hi + lo parts (E = hi + lo to ~2^-17 rel) and
    extended with a ones column; 16 accumulating bf16 matmuls per row-tile
    produce [128 rows, 129] in PSUM = [x@E | row_count] with fp32 accumulation.
  - Epilogue: out = psum[:, :128] * (1 / max(psum[:, 128], 1)).
"""

import sys
from contextlib import ExitStack

import numpy as np

for _p in ("/opt/trn_rl_repo",):
    if _p not in sys.path:
        sys.path.insert(0, _p)

import concourse.bass as bass
import concourse.mybir as mybir
import concourse.tile as tile
from concourse.masks import make_identity

from concourse.vector_clock import ScopedClock


class _SplitDrainTC(tile.TileContext):
    """TileContext whose kernel-tail drain splits its semaphore waits across
    single-wait carrier nops — this walrus build enforces a small
    per-instruction sync-wait limit that the stock all-lane drain exceeds."""

    def _drain_and_barrier(self, tick_clock, wait_clock):
        drain_inst = self.nc.sync.drain()
        wait_clock.add_sem_waits(
            drain_inst.ins, ScopedClock({None: tick_clock.global_clock})
        )
        si = drain_inst.ins.sync_info
        if si is not None and si.on_wait is not None and len(si.on_wait) > 1:
            waits = list(si.on_wait)
            del si.on_wait[1:]
            for w in waits[1:]:
                nop = self.nc.sync.nop(nofuse=True, hint="drain_wait_split")
                nsi = nop.ins.sync_info
                if nsi is None:
                    nop.ins.sync_info = mybir.SyncInfo(on_update=[], on_wait=[w])
                else:
                    nsi.on_wait.append(w)
        self.nc.all_engine_barrier()
        assert self.sems is not None
        popped = self.nc._tile_sem_poison_stack.pop()
        assert popped is self._sem_poison
        self.nc.clear_and_free_semaphores(list(self.sems.allocated().values()))
        self.nc.all_engine_barrier()


B, S, V, D = 128, 200, 1024, 128
NCORES = 8
P = 128
PER_CORE_B = B // NCORES          # 16
ROWS = PER_CORE_B * S             # 3200 rows per core
NCH = V // P                      # 8 vocab chunks
NE = D + 1                        # 128 emb cols + 1 count col


def build_kernel(rows=ROWS, group=5):
    """Build the per-core Bass program. `rows` must be a multiple of 128*group."""
    rt = rows // P                 # row tiles
    assert rt % group == 0
    ng = rt // group               # DMA groups

    nc = bass.Bass()
    x = nc.declare_dram_parameter("x", [rows, V], mybir.dt.float32, isOutput=False)
    emb = nc.declare_dram_parameter("emb", [V, D], mybir.dt.float32, isOutput=False)
    y = nc.declare_dram_parameter("y", [rows, D], mybir.dt.float32, isOutput=True)

    bf16 = mybir.dt.bfloat16
    f32 = mybir.dt.float32

    with _SplitDrainTC(nc) as tc, ExitStack() as ctx:
        const = ctx.enter_context(tc.tile_pool(name="const", bufs=1))
        # one slot per group: avoids slot-reuse waits that push instructions
        # over walrus' one-sync-wait-per-instruction codegen limit
        xb_pool = ctx.enter_context(tc.tile_pool(name="xb", bufs=ng))
        xt_pool = ctx.enter_context(tc.tile_pool(name="xt", bufs=4))
        out_pool = ctx.enter_context(tc.tile_pool(name="out", bufs=ng))
        small = ctx.enter_context(tc.tile_pool(name="small", bufs=4))
        psum_t = ctx.enter_context(tc.tile_pool(name="psum_t", bufs=2, space="PSUM"))
        psum_o = ctx.enter_context(tc.tile_pool(name="psum_o", bufs=2, space="PSUM"))

        # identity for TensorE transposes
        ident = const.tile([P, P], bf16)
        make_identity(nc, ident)

        # E -> bf16 hi/lo split, chunked [p, chunk, d], plus ones/zeros count col
        e_f32 = const.tile([P, NCH, D], f32)
        nc.sync.dma_start(e_f32[:], emb.rearrange("(c p) d -> p c d", p=P))
        rhs_hi = const.tile([P, NCH, NE], bf16)
        rhs_lo = const.tile([P, NCH, NE], bf16)
        e_hi32 = const.tile([P, NCH, D], f32)
        nc.vector.tensor_copy(rhs_hi[:, :, 0:D], e_f32[:])      # round to bf16
        nc.vector.tensor_copy(e_hi32[:], rhs_hi[:, :, 0:D])     # widen back
        nc.vector.tensor_sub(rhs_lo[:, :, 0:D], e_f32[:], e_hi32[:])
        nc.vector.memset(rhs_hi[:, :, D:NE], 1.0)
        nc.vector.memset(rhs_lo[:, :, D:NE], 0.0)

        # row = (g*group + f)*128 + p
        xg = x.rearrange("(g f p) v -> g p f v", p=P, f=group)
        yg = y.rearrange("(g f p) d -> g p f d", p=P, f=group)

        for g in range(ng):
            xb = xb_pool.tile([P, group, V], bf16)
            nc.gpsimd.dma_start(xb[:], xg[g])  # fp32 -> bf16 cast during DMA
            out_sb = out_pool.tile([P, group, D], f32)
            for f in range(group):
                pt = psum_t.tile([P, NCH, P], bf16)
                for c in range(NCH):
                    nc.tensor.transpose(pt[:, c, :], xb[:, f, c * P:(c + 1) * P], ident)
                xt = xt_pool.tile([P, NCH, P], bf16)
                # PSUM -> SBUF copyback on DVE (ACT trips walrus'
                # per-instruction sync-wait limit in this dependency pattern)
                nc.vector.tensor_copy(xt[:, 0:4, :], pt[:, 0:4, :])
                nc.vector.tensor_copy(xt[:, 4:NCH, :], pt[:, 4:NCH, :])
                po = psum_o.tile([P, NE], f32)
                for c in range(NCH):
                    nc.tensor.matmul(po[:], xt[:, c, :], rhs_hi[:, c, :],
                                     start=(c == 0), stop=False)
                    nc.tensor.matmul(po[:], xt[:, c, :], rhs_lo[:, c, :],
                                     start=False, stop=(c == NCH - 1))
                r = small.tile([P, 1], f32)
                nc.vector.tensor_scalar_max(r[:], po[:, D:NE], 1.0)
                nc.vector.reciprocal(r[:], r[:])
                nc.vector.tensor_scalar_mul(out_sb[:, f, :], po[:, 0:D], r[:])
            nc.sync.dma_start(yg[g], out_sb[:])

    return nc


_cached_nc = None


def kernel(**inputs):
    global _cached_nc
    from concourse.bass_utils import run_bass_kernel_spmd

    x = np.asarray(inputs["batch_vectors"], dtype=np.float32).reshape(B, S, V)
    e = np.ascontiguousarray(np.asarray(inputs["embedding_matrix"], dtype=np.float32))

    if _cached_nc is None:
        _cached_nc = build_kernel()

    in_maps = []
    for i in range(NCORES):
        shard = np.ascontiguousarray(
            x[i * PER_CORE_B:(i + 1) * PER_CORE_B].reshape(ROWS, V)
        )
        in_maps.append({"x": shard, "emb": e})

    res = run_bass_kernel_spmd(_cached_nc, in_maps, core_ids=list(range(NCORES)))
    out = np.concatenate(
        [res.results[i]["y"].reshape(PER_CORE_B, S, D) for i in range(NCORES)],
        axis=0,
    )
    return out.astype(np.float32)



# revision 4
# speedup vs baseline: 1.9209x; 1.9209x over previous
"""Trainium2 Bass kernel for nn_KC_Avg_Embedding (multi-hot averaged embedding).

Computes, for multi-hot indicator vectors x[b,s,:] over a vocabulary of 1024:
    out[b,s,:] = (x[b,s,:] @ E) / max(sum(x[b,s,:]), 1)

Strategy (data-parallel over 8 NeuronCores, batch-sharded; memory-regime):
  - Each core gets rows = (B/8)*S = 3200 rows. The host uploads x already
    TRANSPOSED to [V=1024, rows] and cast to fp8e4m3 (x is 0/1 so fp8 is
    exact): 4x less HBM traffic than fp32 and no on-device transposes at all.
  - E is uploaded as fp16 [V, 129] with a ones column appended on host; the
    ones column makes the row-count (the averaging denominator) fall out of
    the same matmuls.
  - Per 128-row tile: 8 accumulating matmuls (one per 128-wide vocab chunk),
    lhsT = x^T chunk (fp8 stationary -> fast weight load), rhs = E_aug fp16
    (moving, N=129), fp32 PSUM -> [128 rows, 129] = [x@E | count].
  - Epilogue: DVE computes r = 1/max(count,1); ACT writes out = psum * r as
    fp16. Output y is fp16 [rows, 128]; host upcasts to fp32.
  - HBM per core: 3.28 MB (x) + 0.26 MB (E) + 0.82 MB (y) = 4.4 MB.
"""

import sys
from contextlib import ExitStack

import numpy as np
import ml_dtypes

for _p in ("/opt/trn_rl_repo",):
    if _p not in sys.path:
        sys.path.insert(0, _p)

import concourse.bass as bass
import concourse.mybir as mybir
import concourse.tile as tile

from concourse.vector_clock import ScopedClock


class _SplitDrainTC(tile.TileContext):
    """TileContext whose kernel-tail drain splits its semaphore waits across
    single-wait carrier nops — this walrus build enforces a small
    per-instruction sync-wait limit that the stock all-lane drain exceeds."""

    def _drain_and_barrier(self, tick_clock, wait_clock):
        drain_inst = self.nc.sync.drain()
        wait_clock.add_sem_waits(
            drain_inst.ins, ScopedClock({None: tick_clock.global_clock})
        )
        si = drain_inst.ins.sync_info
        if si is not None and si.on_wait is not None and len(si.on_wait) > 1:
            waits = list(si.on_wait)
            del si.on_wait[1:]
            for w in waits[1:]:
                nop = self.nc.sync.nop(nofuse=True, hint="drain_wait_split")
                nsi = nop.ins.sync_info
                if nsi is None:
                    nop.ins.sync_info = mybir.SyncInfo(on_update=[], on_wait=[w])
                else:
                    nsi.on_wait.append(w)
        self.nc.all_engine_barrier()
        assert self.sems is not None
        popped = self.nc._tile_sem_poison_stack.pop()
        assert popped is self._sem_poison
        self.nc.clear_and_free_semaphores(list(self.sems.allocated().values()))
        self.nc.all_engine_barrier()


B, S, V, D = 128, 200, 1024, 128
NCORES = 8
P = 128
PER_CORE_B = B // NCORES          # 16
ROWS = PER_CORE_B * S             # 3200 rows per core
NCH = V // P                      # 8 vocab chunks
NE = D + 1                        # 128 emb cols + 1 count col
GROUP = 5                         # row tiles per DMA group
NG = (ROWS // P) // GROUP         # 5 DMA groups
WARMUP_MM = 16                    # dummy matmuls to warm the PE HAM clock gate

FP8 = mybir.dt.float8e4
F16 = mybir.dt.float16
F32 = mybir.dt.float32
NP_FP8 = ml_dtypes.float8_e4m3


def build_kernel():
    nc = bass.Bass()
    # x^T per core: [V, rows] fp8 (host-transposed + cast; 0/1 exact in fp8)
    x = nc.declare_dram_parameter("x", [V, ROWS], FP8, isOutput=False)
    # E augmented with a ones column, fp16, prepared on host: [V, 129]
    emb = nc.declare_dram_parameter("emb", [V, NE], F16, isOutput=False)
    y = nc.declare_dram_parameter("y", [ROWS, D], F16, isOutput=True)

    with _SplitDrainTC(nc) as tc, ExitStack() as ctx:
        const = ctx.enter_context(tc.tile_pool(name="const", bufs=1))
        xb_pool = ctx.enter_context(tc.tile_pool(name="xb", bufs=NG))
        out_pool = ctx.enter_context(tc.tile_pool(name="out", bufs=NG))
        stage_pool = ctx.enter_context(tc.tile_pool(name="stage", bufs=NG))
        small = ctx.enter_context(tc.tile_pool(name="small", bufs=8))
        psum_w = ctx.enter_context(tc.tile_pool(name="psum_w", bufs=1, space="PSUM"))
        psum_o = ctx.enter_context(tc.tile_pool(name="psum_o", bufs=4, space="PSUM"))

        # E_aug -> SBUF, chunked [p, chunk, col]
        rhs = const.tile([P, NCH, NE], F16)
        nc.sync.dma_start(rhs[:], emb.rearrange("(c p) e -> p c e", p=P))

        # Dummy matmuls on zeroed tiles: warm the PE clock gate (HAM) while
        # the first x DMA is still in flight; no data dependencies.
        wz = const.tile([P, P], FP8)
        ez = const.tile([P, NE], F16)
        nc.vector.memset(wz[:], 0.0)
        nc.vector.memset(ez[:], 0.0)
        pw = psum_w.tile([P, NE], F32)
        for _ in range(WARMUP_MM):
            nc.tensor.matmul(pw[:], wz[:], ez[:], start=True, stop=True)

        # x^T HBM [V, rows]: v = c*128 + p; rows split into NG column groups
        xg = x.rearrange("(c p) (g r) -> g p c r", p=P, g=NG)
        yg = y.rearrange("(g f p) d -> g p f d", p=P, f=GROUP)

        for g in range(NG):
            xb = xb_pool.tile([P, NCH, GROUP * P], FP8)
            nc.sync.dma_start(xb[:], xg[g])
            # ACT stages each PSUM tile to SBUF (single sync-wait on the PE);
            # DVE then does the whole group's normalization from SBUF.
            stage = stage_pool.tile([P, GROUP, NE], F32)
            for f in range(GROUP):
                po = psum_o.tile([P, NE], F32)
                for c in range(NCH):
                    nc.tensor.matmul(po[:], xb[:, c, f * P:(f + 1) * P],
                                     rhs[:, c, :],
                                     start=(c == 0), stop=(c == NCH - 1))
                nc.scalar.copy(stage[:, f, :], po[:])
            r = small.tile([P, GROUP], F32)
            nc.vector.tensor_scalar_max(r[:], stage[:, :, D], 1.0)
            nc.vector.reciprocal(r[:], r[:])
            out_sb = out_pool.tile([P, GROUP, D], F16)
            for f in range(GROUP):
                nc.vector.tensor_scalar_mul(out_sb[:, f, :], stage[:, f, 0:D],
                                            r[:, f:f + 1])
            nc.gpsimd.dma_start(yg[g], out_sb[:])

    return nc


_cached_nc = None


def make_in_maps(batch_vectors, embedding_matrix):
    """Host-side prep: shard + transpose + cast. Pure layout/dtype changes."""
    x = np.asarray(batch_vectors, dtype=np.float32).reshape(B, S, V)
    e = np.asarray(embedding_matrix, dtype=np.float32)
    e_aug = np.empty((V, NE), dtype=np.float16)
    e_aug[:, 0:D] = e.astype(np.float16)
    e_aug[:, D] = np.float16(1.0)
    in_maps = []
    for i in range(NCORES):
        shard = x[i * PER_CORE_B:(i + 1) * PER_CORE_B].reshape(ROWS, V)
        xt = np.ascontiguousarray(shard.T).astype(NP_FP8)
        in_maps.append({"x": xt, "emb": e_aug})
    return in_maps


def kernel(**inputs):
    global _cached_nc
    from concourse.bass_utils import run_bass_kernel_spmd

    if _cached_nc is None:
        _cached_nc = build_kernel()

    in_maps = make_in_maps(inputs["batch_vectors"], inputs["embedding_matrix"])
    res = run_bass_kernel_spmd(_cached_nc, in_maps, core_ids=list(range(NCORES)))
    out = np.concatenate(
        [
            res.results[i]["y"].astype(np.float32).reshape(PER_CORE_B, S, D)
            for i in range(NCORES)
        ],
        axis=0,
    )
    return out


# revision 9
# speedup vs baseline: 2.0537x; 1.0691x over previous
"""Trainium2 Bass kernel for nn_KC_Avg_Embedding (multi-hot averaged embedding).

Computes, for multi-hot indicator vectors x[b,s,:] over a vocabulary of 1024:
    out[b,s,:] = (x[b,s,:] @ E) / max(sum(x[b,s,:]), 1)

Strategy (data-parallel over 8 NeuronCores, batch-sharded; memory-regime):
  - Each core gets rows = (B/8)*S = 3200 rows. The host uploads x already
    TRANSPOSED, swizzled into the exact SBUF tile layout [p, chunk, row] per
    DMA group, and cast to fp8e4m3 (x is 0/1 so fp8 is exact): 4x less HBM
    traffic than fp32, no on-device transposes, and every DMA descriptor is
    a contiguous multi-KB run per partition.
  - E is uploaded as fp16 [p, chunk, 129] with a ones column appended on
    host; the ones column makes the row-count (averaging denominator) fall
    out of the same matmuls.
  - Per 128-row tile: 8 accumulating matmuls (one per 128-wide vocab chunk),
    lhsT = x^T chunk (fp8 stationary -> fast weight load), rhs = E_aug fp16
    (moving, N=129), fp32 PSUM -> [128 rows, 129] = [x@E | count].
  - Epilogue: ACT stages PSUM->SBUF (single sync-wait), DVE computes
    r = 1/max(count,1) and scales, writing fp16; scalar-engine HWDGE ring
    DMAs the fp16 output out. Host upcasts to fp32.
  - Row-tile groups are sized [2,3,5,5,5,3,2]: small first group so matmuls
    start as early as possible, small last group so the drain tail is short.
  - Dummy matmuls on zeroed tiles warm the PE HAM clock gate during the
    initial DMA; HBM per core: 3.28 MB (x) + 0.26 MB (E) + 0.82 MB (y).
"""

import sys
from contextlib import ExitStack

import numpy as np
import ml_dtypes

for _p in ("/opt/trn_rl_repo",):
    if _p not in sys.path:
        sys.path.insert(0, _p)

import concourse.bass as bass
import concourse.mybir as mybir
import concourse.tile as tile

from concourse.vector_clock import ScopedClock


class _SplitDrainTC(tile.TileContext):
    """TileContext whose kernel-tail drain splits its semaphore waits across
    single-wait carrier nops — this walrus build enforces a small
    per-instruction sync-wait limit that the stock all-lane drain exceeds."""

    def _drain_and_barrier(self, tick_clock, wait_clock):
        drain_inst = self.nc.sync.drain()
        wait_clock.add_sem_waits(
            drain_inst.ins, ScopedClock({None: tick_clock.global_clock})
        )
        si = drain_inst.ins.sync_info
        if si is not None and si.on_wait is not None and len(si.on_wait) > 1:
            waits = list(si.on_wait)
            del si.on_wait[1:]
            for w in waits[1:]:
                nop = self.nc.sync.nop(nofuse=True, hint="drain_wait_split")
                nsi = nop.ins.sync_info
                if nsi is None:
                    nop.ins.sync_info = mybir.SyncInfo(on_update=[], on_wait=[w])
                else:
                    nsi.on_wait.append(w)
        self.nc.all_engine_barrier()
        assert self.sems is not None
        popped = self.nc._tile_sem_poison_stack.pop()
        assert popped is self._sem_poison
        self.nc.clear_and_free_semaphores(list(self.sems.allocated().values()))
        self.nc.all_engine_barrier()


B, S, V, D = 128, 200, 1024, 128
NCORES = 8
P = 128
PER_CORE_B = B // NCORES          # 16
ROWS = PER_CORE_B * S             # 3200 rows per core
NCH = V // P                      # 8 vocab chunks
NE = D + 1                        # 128 emb cols + 1 count col
GSIZES = [2, 3, 5, 5, 5, 3, 2]    # row tiles per DMA group (sum = 25)
NG = len(GSIZES)
WARMUP_MM = 20                    # dummy matmuls to warm the PE HAM clock gate

FP8 = mybir.dt.float8e4
F16 = mybir.dt.float16
F32 = mybir.dt.float32
NP_FP8 = ml_dtypes.float8_e4m3
FP8_ONE = 0x38                    # bit pattern of 1.0 in fp8e4m3

assert sum(GSIZES) * P == ROWS


def build_kernel():
    nc = bass.Bass()
    # x^T per group, pre-swizzled on host to [p, chunk, row]: fp8, 0/1 exact
    xs = [
        nc.declare_dram_parameter(f"x{g}", [P, NCH, GSIZES[g] * P], FP8,
                                  isOutput=False)
        for g in range(NG)
    ]
    # E augmented with ones column, fp16, host-swizzled to [p, chunk, col]
    emb = nc.declare_dram_parameter("emb", [P, NCH, NE], F16, isOutput=False)
    y = nc.declare_dram_parameter("y", [ROWS, D], F16, isOutput=True)

    with _SplitDrainTC(nc) as tc, ExitStack() as ctx:
        const = ctx.enter_context(tc.tile_pool(name="const", bufs=1))
        xb_pool = ctx.enter_context(tc.tile_pool(name="xb", bufs=NG))
        out_pool = ctx.enter_context(tc.tile_pool(name="out", bufs=NG))
        stage_pool = ctx.enter_context(tc.tile_pool(name="stage", bufs=NG))
        small = ctx.enter_context(tc.tile_pool(name="small", bufs=NG))
        psum_w = ctx.enter_context(tc.tile_pool(name="psum_w", bufs=1, space="PSUM"))
        psum_o = ctx.enter_context(tc.tile_pool(name="psum_o", bufs=4, space="PSUM"))

        rhs = const.tile([P, NCH, NE], F16)
        nc.sync.dma_start(rhs[:], emb[:])

        # Dummy matmuls on zeroed tiles: warm the PE clock gate (HAM) while
        # the first x DMA is in flight. memsets on GpSimd so the PE isn't
        # gated on DVE's preamble table loads.
        wz = const.tile([P, P], FP8)
        ez = const.tile([P, NE], F16)
        dummy = const.tile([P, NG], F16)
        nc.gpsimd.memset(wz[:], 0.0)
        nc.gpsimd.memset(ez[:], 0.0)
        pw = psum_w.tile([P, NE], F32)
        for _ in range(WARMUP_MM):
            nc.tensor.matmul(pw[:], wz[:], ez[:], start=True, stop=True)

        # y rows are tile-major: row = t*128 + p
        yt = y.rearrange("(t p) d -> p t d", p=P)

        t0 = 0
        for g, sz in enumerate(GSIZES):
            xb = xb_pool.tile([P, NCH, sz * P], FP8)
            nc.sync.dma_start(xb[:], xs[g][:])
            # ACT stages each PSUM tile to SBUF (single sync-wait on the PE);
            # DVE then does the whole group's normalization from SBUF.
            stage = stage_pool.tile([P, sz, NE], F32)
            for f in range(sz):
                po = psum_o.tile([P, NE], F32)
                for c in range(NCH):
                    nc.tensor.matmul(po[:], xb[:, c, f * P:(f + 1) * P],
                                     rhs[:, c, :],
                                     start=(c == 0), stop=(c == NCH - 1))
                nc.scalar.copy(stage[:, f, :], po[:])
            r = small.tile([P, sz], F32)
            nc.vector.tensor_scalar_max(r[:], stage[:, :, D], 1.0)
            nc.vector.reciprocal(r[:], r[:])
            out_sb = out_pool.tile([P, sz, D], F16)
            for f in range(sz):
                nc.vector.tensor_scalar_mul(out_sb[:, f, :], stage[:, f, 0:D],
                                            r[:, f:f + 1])
            # output on gpsimd (SWDGE): tolerates the multi-sem-wait this
            # join needs (HWDGE DMA instructions are limited to one wait)
            nc.gpsimd.dma_start(yt[:, t0:t0 + sz, :], out_sb[:])
            t0 += sz

    return nc


_cached_nc = None


def make_in_maps(batch_vectors, embedding_matrix):
    """Host-side prep: shard + transpose + swizzle + cast. Layout/dtype only."""
    x = np.asarray(batch_vectors, dtype=np.float32).reshape(B, S, V)
    e = np.asarray(embedding_matrix, dtype=np.float32)
    e_aug = np.empty((V, NE), dtype=np.float16)
    e_aug[:, 0:D] = e.astype(np.float16)
    e_aug[:, D] = np.float16(1.0)
    # [V, NE] -> [p, chunk, NE]
    e_dev = np.ascontiguousarray(e_aug.reshape(NCH, P, NE).transpose(1, 0, 2))

    # 0/1 -> fp8 bit pattern, then pure reshape/transpose per group
    xb = (x != 0).astype(np.uint8) * np.uint8(FP8_ONE)
    in_maps = []
    for i in range(NCORES):
        shard = xb[i * PER_CORE_B:(i + 1) * PER_CORE_B].reshape(ROWS, V)
        m = {"emb": e_dev}
        t0 = 0
        for g, sz in enumerate(GSIZES):
            blk = shard[t0 * P:(t0 + sz) * P, :].T        # [V, sz*P]
            blk = blk.reshape(NCH, P, sz * P).transpose(1, 0, 2)
            m[f"x{g}"] = np.ascontiguousarray(blk).view(NP_FP8)
            t0 += sz
        in_maps.append(m)
    return in_maps


def kernel(**inputs):
    global _cached_nc
    from concourse.bass_utils import run_bass_kernel_spmd

    if _cached_nc is None:
        _cached_nc = build_kernel()

    in_maps = make_in_maps(inputs["batch_vectors"], inputs["embedding_matrix"])
    res = run_bass_kernel_spmd(_cached_nc, in_maps, core_ids=list(range(NCORES)))
    out = np.concatenate(
        [
            res.results[i]["y"].astype(np.float32).reshape(PER_CORE_B, S, D)
            for i in range(NCORES)
        ],
        axis=0,
    )
    return out


# revision 11
# speedup vs baseline: 2.1093x; 1.0271x over previous
"""Trainium2 Bass kernel for nn_KC_Avg_Embedding (multi-hot averaged embedding).

Computes, for multi-hot indicator vectors x[b,s,:] over a vocabulary of 1024:
    out[b,s,:] = (x[b,s,:] @ E) / max(sum(x[b,s,:]), 1)

Strategy (data-parallel over 8 NeuronCores, batch-sharded; memory-regime):
  - Each core gets rows = (B/8)*S = 3200 rows. The host uploads x already
    TRANSPOSED, swizzled into the exact SBUF tile layout [p, chunk, row] per
    DMA group, and cast to fp8e4m3 (x is 0/1 so fp8 is exact): 4x less HBM
    traffic than fp32, no on-device transposes, and every DMA descriptor is
    a contiguous multi-KB run per partition.
  - E is uploaded as fp16 [p, chunk, 129] with a ones column appended on
    host; the ones column makes the row-count (averaging denominator) fall
    out of the same matmuls.
  - Per 128-row tile: 8 accumulating matmuls (one per 128-wide vocab chunk),
    lhsT = x^T chunk (fp8 stationary -> fast weight load), rhs = E_aug fp16
    (moving, N=129), fp32 PSUM -> [128 rows, 129] = [x@E | count].
  - Epilogue: ACT stages PSUM->SBUF (single sync-wait), DVE computes
    r = 1/max(count,1) and scales, writing fp16; scalar-engine HWDGE ring
    DMAs the fp16 output out. Host upcasts to fp32.
  - Row-tile groups are sized [2,3,5,5,5,3,2]: small first group so matmuls
    start as early as possible, small last group so the drain tail is short.
  - Dummy matmuls on zeroed tiles warm the PE HAM clock gate during the
    initial DMA; HBM per core: 3.28 MB (x) + 0.26 MB (E) + 0.82 MB (y).
"""

import sys
from contextlib import ExitStack

import numpy as np
import ml_dtypes

for _p in ("/opt/trn_rl_repo",):
    if _p not in sys.path:
        sys.path.insert(0, _p)

import concourse.bass as bass
import concourse.mybir as mybir
import concourse.tile as tile

from concourse.vector_clock import ScopedClock


class _SplitDrainTC(tile.TileContext):
    """TileContext whose kernel-tail drain splits its semaphore waits across
    single-wait carrier nops — this walrus build enforces a small
    per-instruction sync-wait limit that the stock all-lane drain exceeds."""

    def _drain_and_barrier(self, tick_clock, wait_clock):
        drain_inst = self.nc.sync.drain()
        wait_clock.add_sem_waits(
            drain_inst.ins, ScopedClock({None: tick_clock.global_clock})
        )
        si = drain_inst.ins.sync_info
        if si is not None and si.on_wait is not None and len(si.on_wait) > 1:
            waits = list(si.on_wait)
            del si.on_wait[1:]
            for w in waits[1:]:
                nop = self.nc.sync.nop(nofuse=True, hint="drain_wait_split")
                nsi = nop.ins.sync_info
                if nsi is None:
                    nop.ins.sync_info = mybir.SyncInfo(on_update=[], on_wait=[w])
                else:
                    nsi.on_wait.append(w)
        self.nc.all_engine_barrier()
        assert self.sems is not None
        popped = self.nc._tile_sem_poison_stack.pop()
        assert popped is self._sem_poison
        self.nc.clear_and_free_semaphores(list(self.sems.allocated().values()))
        self.nc.all_engine_barrier()


B, S, V, D = 128, 200, 1024, 128
NCORES = 8
P = 128
PER_CORE_B = B // NCORES          # 16
ROWS = PER_CORE_B * S             # 3200 rows per core
NCH = V // P                      # 8 vocab chunks
NE = D + 1                        # 128 emb cols + 1 count col
GSIZES = [2, 3, 5, 5, 5, 3, 2]    # row tiles per DMA group (sum = 25)
NG = len(GSIZES)
WARMUP_MM = 30                    # dummy matmuls to warm the PE HAM clock gate

FP8 = mybir.dt.float8e4
F16 = mybir.dt.float16
F32 = mybir.dt.float32
NP_FP8 = ml_dtypes.float8_e4m3
FP8_ONE = 0x38                    # bit pattern of 1.0 in fp8e4m3

assert sum(GSIZES) * P == ROWS


def build_kernel():
    nc = bass.Bass()
    # x^T per group, pre-swizzled on host to [p, chunk, row]: fp8, 0/1 exact
    xs = [
        nc.declare_dram_parameter(f"x{g}", [P, NCH, GSIZES[g] * P], FP8,
                                  isOutput=False)
        for g in range(NG)
    ]
    # E augmented with ones column, fp16, host-swizzled to [p, chunk, col]
    emb = nc.declare_dram_parameter("emb", [P, NCH, NE], F16, isOutput=False)
    y = nc.declare_dram_parameter("y", [ROWS, D], F16, isOutput=True)

    with _SplitDrainTC(nc) as tc, ExitStack() as ctx:
        const = ctx.enter_context(tc.tile_pool(name="const", bufs=1))
        xb_pool = ctx.enter_context(tc.tile_pool(name="xb", bufs=NG))
        out_pool = ctx.enter_context(tc.tile_pool(name="out", bufs=NG))
        stage_pool = ctx.enter_context(tc.tile_pool(name="stage", bufs=NG))
        small = ctx.enter_context(tc.tile_pool(name="small", bufs=NG))
        psum_w = ctx.enter_context(tc.tile_pool(name="psum_w", bufs=1, space="PSUM"))
        psum_o = ctx.enter_context(tc.tile_pool(name="psum_o", bufs=4, space="PSUM"))

        # Input DMAs alternate between the two HWDGE rings (Sync + Scalar):
        # each DMA_DIRECT2D occupies its sequencer ~0.65us generating
        # descriptors, so one ring would serialize ~5us of issue latency.
        # x0 goes first (smallest group -> earliest first matmul), emb in
        # parallel on the other ring.
        xb_tiles = []
        for g, sz in enumerate(GSIZES):
            xb = xb_pool.tile([P, NCH, sz * P], FP8)
            xb_tiles.append(xb)
        rhs = const.tile([P, NCH, NE], F16)
        nc.sync.dma_start(xb_tiles[0][:], xs[0][:])
        nc.scalar.dma_start(rhs[:], emb[:])
        for g in range(1, NG):
            eng = nc.sync if g % 2 == 1 else nc.scalar
            eng.dma_start(xb_tiles[g][:], xs[g][:])

        # Dummy matmuls on zeroed tiles: warm the PE clock gate (HAM) and
        # keep it busy until the first x group lands; memsets on DVE (fast
        # launch, idle early).
        wz = const.tile([P, P], FP8)
        ez = const.tile([P, NE], F16)
        nc.vector.memset(wz[:], 0.0)
        nc.vector.memset(ez[:], 0.0)
        pw = psum_w.tile([P, NE], F32)
        for _ in range(WARMUP_MM):
            nc.tensor.matmul(pw[:], wz[:], ez[:], start=True, stop=True)

        # y rows are tile-major: row = t*128 + p
        yt = y.rearrange("(t p) d -> p t d", p=P)

        t0 = 0
        for g, sz in enumerate(GSIZES):
            xb = xb_tiles[g]
            # ACT stages each PSUM tile to SBUF (single sync-wait on the PE);
            # DVE then does the whole group's normalization from SBUF.
            stage = stage_pool.tile([P, sz, NE], F32)
            for f in range(sz):
                po = psum_o.tile([P, NE], F32)
                for c in range(NCH):
                    nc.tensor.matmul(po[:], xb[:, c, f * P:(f + 1) * P],
                                     rhs[:, c, :],
                                     start=(c == 0), stop=(c == NCH - 1))
                nc.scalar.copy(stage[:, f, :], po[:])
            r = small.tile([P, sz], F32)
            nc.vector.tensor_scalar_max(r[:], stage[:, :, D], 1.0)
            nc.vector.reciprocal(r[:], r[:])
            out_sb = out_pool.tile([P, sz, D], F16)
            for f in range(sz):
                nc.vector.tensor_scalar_mul(out_sb[:, f, :], stage[:, f, 0:D],
                                            r[:, f:f + 1])
            # output on gpsimd (SWDGE): tolerates the multi-sem-wait this
            # join needs (HWDGE DMA instructions are limited to one wait)
            nc.gpsimd.dma_start(yt[:, t0:t0 + sz, :], out_sb[:])
            t0 += sz

    return nc


_cached_nc = None


def make_in_maps(batch_vectors, embedding_matrix):
    """Host-side prep: shard + transpose + swizzle + cast. Layout/dtype only."""
    x = np.asarray(batch_vectors, dtype=np.float32).reshape(B, S, V)
    e = np.asarray(embedding_matrix, dtype=np.float32)
    e_aug = np.empty((V, NE), dtype=np.float16)
    e_aug[:, 0:D] = e.astype(np.float16)
    e_aug[:, D] = np.float16(1.0)
    # [V, NE] -> [p, chunk, NE]
    e_dev = np.ascontiguousarray(e_aug.reshape(NCH, P, NE).transpose(1, 0, 2))

    # 0/1 -> fp8 bit pattern, then pure reshape/transpose per group
    xb = (x != 0).astype(np.uint8) * np.uint8(FP8_ONE)
    in_maps = []
    for i in range(NCORES):
        shard = xb[i * PER_CORE_B:(i + 1) * PER_CORE_B].reshape(ROWS, V)
        m = {"emb": e_dev}
        t0 = 0
        for g, sz in enumerate(GSIZES):
            blk = shard[t0 * P:(t0 + sz) * P, :].T        # [V, sz*P]
            blk = blk.reshape(NCH, P, sz * P).transpose(1, 0, 2)
            m[f"x{g}"] = np.ascontiguousarray(blk).view(NP_FP8)
            t0 += sz
        in_maps.append(m)
    return in_maps


def kernel(**inputs):
    global _cached_nc
    from concourse.bass_utils import run_bass_kernel_spmd

    if _cached_nc is None:
        _cached_nc = build_kernel()

    in_maps = make_in_maps(inputs["batch_vectors"], inputs["embedding_matrix"])
    res = run_bass_kernel_spmd(_cached_nc, in_maps, core_ids=list(range(NCORES)))
    out = np.concatenate(
        [
            res.results[i]["y"].astype(np.float32).reshape(PER_CORE_B, S, D)
            for i in range(NCORES)
        ],
        axis=0,
    )
    return out
